# revision 44
# baseline (speedup 1.0000x reference)
"""Trainium2 Bass kernel for nn_BaseLoftqLinear (4-bit quantized linear + LoRA + bias).

Computes: out = x @ dequant(W).T + (x @ A.T) @ B.T + bias
  x: [4, 2048, 4096] f32, W: [4096, 4096] 4-bit packed, A: [16, 4096], B: [4096, 16]

Strategy (column-parallel / tensor-parallel over out_features across 8 cores):
  - each core owns 512 out_features: its shard of packed_qweight/weight_max/lora_B/bias
  - x and lora_A replicated; x is converted to bf16 on HOST so the device can
    use the xbar DMA-transpose to load x tiles directly in [K, M] layout --
    this removes all 2048 PE transposes + PSUM->SBUF copies from the hot loop
    and halves x HBM traffic
  - packed_qweight is unpacked to one nibble per byte on HOST (pure bit
    marshaling): the device dequant is then 3 wide DVE ops per tile instead
    of 4 narrow ones, with no strided half-interleaving
  - on device: dequantize W shard, fuse lora via W_eff = W + B@A (bf16 PE
    matmul, staged into SBUF via scalar-engine copies), single big GEMM
    x @ W_eff.T in bf16, add bias
  - host gathers the 8 [8192, 512] outputs -> [4, 2048, 4096]
"""
import os
import sys

for _p in ("/opt/trn_rl_repo", "/root/.axon_site/_ro/trn_rl_repo"):
    if os.path.isdir(_p) and _p not in sys.path:
        sys.path.insert(0, _p)
        break

import numpy as np
import ml_dtypes

import concourse.bass as bass
import concourse.bacc as bacc
import concourse.tile as tile
import concourse.mybir as mybir

dt = mybir.dt

# problem constants (hardcoded per spec)
B_, S_, IN_F, OUT_F, RANK = 4, 2048, 4096, 4096, 16
N_CORES = 8
M = B_ * S_                    # 8192 tokens
N = OUT_F // N_CORES           # 512 out_features per core
BLOCK = 64                     # quant block size (along in_features)
OT = N // 128                  # 4 o-tiles per core
MT = M // 128                  # 64 m-tiles
KC = IN_F // 128               # 32 k-chunks


def build_program(affine: bool, c1: float, delta: float, lut_vals):
    """Build the single-core Bass program (SPMD: same program on all 8 cores)."""
    nc = bacc.Bacc("TRN2", target_bir_lowering=False, debug=False,
                   num_devices=N_CORES)

    x = nc.dram_tensor("x", [M, IN_F], dt.bfloat16, kind="ExternalInput")
    pk = nc.dram_tensor("pk", [N, IN_F], dt.uint8, kind="ExternalInput")
    wmax = nc.dram_tensor("wmax", [N, BLOCK], dt.float32, kind="ExternalInput")
    lora_a = nc.dram_tensor("lora_a", [RANK, IN_F], dt.bfloat16, kind="ExternalInput")
    lora_bt = nc.dram_tensor("lora_bt", [RANK, N], dt.bfloat16, kind="ExternalInput")
    bias = nc.dram_tensor("bias", [N], dt.float32, kind="ExternalInput")
    ident = nc.dram_tensor("ident", [128, 128], dt.bfloat16, kind="ExternalInput")
    out = nc.dram_tensor("out", [M, N], dt.float32, kind="ExternalOutput")

    with tile.TileContext(nc) as tc:
        with (
            tc.tile_pool(name="const", bufs=1) as constp,
            tc.tile_pool(name="wprep", bufs=2) as wprep,
            tc.tile_pool(name="wsb", bufs=4) as wsbp,
            tc.tile_pool(name="wt", bufs=1) as wtp,
            tc.tile_pool(name="xt", bufs=3) as xtp,
            tc.tile_pool(name="osb", bufs=3) as op_,
            tc.tile_pool(name="ps_ba", bufs=2, space="PSUM") as ps_ba,
            tc.tile_pool(name="ps_tx", bufs=2, space="PSUM") as ps_tx,
            tc.tile_pool(name="ps_out", bufs=3, space="PSUM") as ps_out,
        ):
            # ---- constants / W inputs on the sync (SP) HWDGE queue.
            # Queue is FIFO: these loads run first, then the 16 xbar
            # transpose windows of x stream behind them.
            id_sb = constp.tile([128, 128], dt.bfloat16, name="id_sb")
            nc.sync.dma_start(out=id_sb[:], in_=ident[:, :])
            a_sb = constp.tile([RANK, IN_F], dt.bfloat16, name="a_sb")
            nc.sync.dma_start(out=a_sb[:], in_=lora_a[:, :])
            bt_sb = constp.tile([RANK, N], dt.bfloat16, name="bt_sb")
            nc.sync.dma_start(out=bt_sb[:], in_=lora_bt[:, :])
            bias_sb = constp.tile([128, N], dt.float32, name="bias_sb")
            bsrc = bass.AP(bias[:].tensor, 0, [[0, 128], [1, N]])
            nc.gpsimd.dma_start(out=bias_sb[:], in_=bsrc)

            nibs, s1s = [], []
            for t in range(OT):
                nib = constp.tile([128, IN_F], dt.uint8, name=f"nib{t}")
                nc.sync.dma_start(out=nib[:], in_=pk[t * 128:(t + 1) * 128, :])
                nibs.append(nib)
                s1 = constp.tile([128, BLOCK], dt.float32, name=f"s1_{t}")
                nc.sync.dma_start(out=s1[:], in_=wmax[t * 128:(t + 1) * 128, :])
                if affine:
                    nc.vector.tensor_scalar_mul(s1[:], s1[:], float(c1))
                s1s.append(s1)

            # ---- W-prep ----
            # wt_sb[:, c*N + t*128 + oo] = W_eff[t*128+oo, c*128+p] on partition p
            # Chunk-major over 512-wide k segments so wt chunk 0 is ready
            # ~1/8 into W-prep and the main matmul loop starts almost
            # immediately.
            wt_sb = wtp.tile([128, KC * N], dt.bfloat16, name="wt_sb")
            w_sbs = []
            for t in range(OT):
                w_sb = wsbp.tile([128, IN_F], dt.bfloat16, tag="w_sb")
                w_sbs.append(w_sb)

            SW = 512
            for k0 in range(0, IN_F, SW):
                for t in range(OT):
                    nib, s1, w_sb = nibs[t], s1s[t], w_sbs[t]
                    # lora B@A for this [128 o, 512 in] segment (bf16, K=16),
                    # staged PSUM -> SBUF bf16 by the (otherwise idle) scalar
                    # engine so the DVE add below is an all-bf16 packed op
                    pba = ps_ba.tile([128, SW], dt.float32, tag="pba")
                    nc.tensor.matmul(
                        pba[:], bt_sb[:, t * 128:(t + 1) * 128],
                        a_sb[:, k0:k0 + SW],
                        start=True, stop=True,
                    )
                    # stage B@A straight into the W_eff segment; the dequant
                    # result is added in place below
                    ba_seg = w_sb[:, k0:k0 + SW]
                    nc.scalar.activation(
                        ba_seg, pba[:],
                        mybir.ActivationFunctionType.Copy, bias=0.0)

                    ns = nib[:, k0:k0 + SW]
                    tl = wprep.tile([128, SW], dt.bfloat16, tag="deq_t")
                    if affine:
                        # nibble + delta (exact in bf16: halves in [-8, 8))
                        nc.vector.tensor_scalar(
                            tl[:], ns, float(delta), None,
                            mybir.AluOpType.add)
                    else:
                        # general 16-entry codebook fallback:
                        # idx -> sum_k lut[k] * (idx == k)
                        nc.vector.memset(tl[:], 0.0)
                        for k in range(16):
                            msk = wprep.tile([128, SW], dt.float32, tag="deq_msk")
                            nc.vector.tensor_scalar(
                                msk[:], ns, float(k), None,
                                mybir.AluOpType.is_equal,
                            )
                            nc.vector.tensor_scalar_mul(
                                msk[:], msk[:], float(lut_vals[k]))
                            nc.vector.tensor_tensor(
                                tl[:], tl[:], msk[:], mybir.AluOpType.add)
                    # multiply by per-block scale (broadcast 64 elems/block)
                    s_ap0 = s1[:]
                    s_b = bass.AP(
                        s_ap0.tensor, s_ap0.offset + k0 // BLOCK,
                        [list(s_ap0.ap[0]), [1, SW // BLOCK], [0, BLOCK]],
                    )
                    nc.vector.tensor_tensor(tl[:], tl[:], s_b, mybir.AluOpType.mult)
                    # add dequant onto the staged B@A in place (all-bf16 packed)
                    nc.vector.tensor_tensor(
                        ba_seg, tl[:], ba_seg, mybir.AluOpType.add)

                # transpose W_eff for the 4 k-chunks this segment completed
                for c in range(k0 // 128, (k0 + SW) // 128):
                    ptr = ps_tx.tile([128, N], dt.bfloat16, tag="ptx")
                    for t in range(OT):
                        nc.tensor.transpose(
                            ptr[:, t * 128:(t + 1) * 128],
                            w_sbs[t][:, c * 128:(c + 1) * 128],
                            id_sb[:],
                        )
                    nc.scalar.activation(
                        wt_sb[:, c * N:(c + 1) * N], ptr[:],
                        mybir.ActivationFunctionType.Copy, bias=0.0)

            # ---- main loop: xbar-transpose-load x, matmul, bias, store ----
            # 4 m-tiles per xbar instruction: the sync queue spends ~5us of
            # fixed overhead (sems + dispatch) per DMA_TRANSPOSE on top of
            # ~14ns/tile processing, so per-m-tile transposes can only supply
            # one tile per ~10us and starve the PE (8us/m-tile). Keep the
            # windows uniform: mixed-size xt tiles break the pool ring and
            # cost ~100us.
            windows = [4] * 16
            assert sum(windows) == MT
            ms0 = 0
            for MW in windows:
                xt = xtp.tile([128, KC, MW * 128], dt.bfloat16, tag="xt")
                nc.sync.dma_start_transpose(
                    out=xt[:], in_=x[ms0 * 128:(ms0 + MW) * 128, :])

                for i in range(MW):
                    ms = ms0 + i
                    po = ps_out.tile([128, N], dt.float32, tag="po")
                    for c in range(KC):
                        nc.tensor.matmul(
                            po[:],
                            xt[:, c, i * 128:(i + 1) * 128],
                            wt_sb[:, c * N:(c + 1) * N],
                            start=(c == 0), stop=(c == KC - 1),
                        )
                    o_sb = op_.tile([128, N], dt.float32, tag="o_sb")
                    nc.vector.tensor_tensor(
                        o_sb[:], po[:], bias_sb[:], mybir.AluOpType.add)
                    nc.scalar.dma_start(
                        out=out[ms * 128:(ms + 1) * 128, :], in_=o_sb[:])
                ms0 += MW

    nc.compile()
    return nc


_cache = {}


def _get_program(lut: np.ndarray):
    lut = np.asarray(lut, dtype=np.float32)
    c1 = float(lut[15] - lut[0]) / 15.0
    idx = np.arange(16, dtype=np.float32)
    affine = bool(
        np.max(np.abs(lut - (lut[0] + c1 * idx))) <= 1e-6 * max(1e-30, np.max(np.abs(lut)))
        and abs(c1) > 1e-20
    )
    delta = float(lut[0]) / c1 if affine else 0.0
    key = (affine, round(c1, 12), round(delta, 12), tuple(np.round(lut, 10).tolist()))
    if key not in _cache:
        _cache[key] = build_program(affine, c1, delta, lut.tolist())
    return _cache[key]


def make_in_maps(inputs: dict):
    x = np.ascontiguousarray(
        np.asarray(inputs["x"], dtype=np.float32).reshape(M, IN_F)
    ).astype(ml_dtypes.bfloat16)
    pk_full = np.asarray(inputs["packed_qweight"]).astype(np.uint8).reshape(-1)
    # unpack to one nibble per byte (LSB-first order, matching the reference)
    nib_full = np.empty(pk_full.size * 2, dtype=np.uint8)
    nib_full[0::2] = pk_full & 15
    nib_full[1::2] = pk_full >> 4
    nib_full = nib_full.reshape(OUT_F, IN_F)
    wmax_full = np.asarray(inputs["weight_max"], dtype=np.float32).reshape(-1)
    lora_a = np.ascontiguousarray(
        np.asarray(inputs["lora_A"], dtype=np.float32)).astype(ml_dtypes.bfloat16)
    lora_b = np.asarray(inputs["lora_B"], dtype=np.float32)
    bias_full = np.asarray(inputs["bias"], dtype=np.float32).reshape(-1)
    ident = np.eye(128, dtype=ml_dtypes.bfloat16)

    in_maps = []
    for i in range(N_CORES):
        o0, o1 = i * N, (i + 1) * N
        in_maps.append({
            "x": x,
            "pk": nib_full[o0:o1],
            "wmax": wmax_full[o0 * BLOCK: o1 * BLOCK].reshape(N, BLOCK),
            "lora_a": lora_a,
            "lora_bt": np.ascontiguousarray(lora_b[o0:o1].T).astype(ml_dtypes.bfloat16),
            "bias": bias_full[o0:o1],
            "ident": ident,
        })
    return in_maps


def kernel(**inputs) -> np.ndarray:
    from concourse.bass_utils import run_bass_kernel_spmd

    nc = _get_program(inputs["lookup_table"])
    in_maps = make_in_maps(inputs)
    res = run_bass_kernel_spmd(nc, in_maps, core_ids=list(range(N_CORES)))
    outs = [np.asarray(r["out"], dtype=np.float32) for r in res.results]
    full = np.concatenate(outs, axis=1)  # [M, OUT_F]
    return full.reshape(B_, S_, OUT_F)


# revision 46
# speedup vs baseline: 1.0074x; 1.0074x over previous
"""Trainium2 Bass kernel for nn_BaseLoftqLinear (4-bit quantized linear + LoRA + bias).

Computes: out = x @ dequant(W).T + (x @ A.T) @ B.T + bias
  x: [4, 2048, 4096] f32, W: [4096, 4096] 4-bit packed, A: [16, 4096], B: [4096, 16]

Strategy (column-parallel / tensor-parallel over out_features across 8 cores):
  - each core owns 512 out_features: its shard of packed_qweight/weight_max/lora_B/bias
  - x and lora_A replicated; x is converted to bf16 on HOST so the device can
    use the xbar DMA-transpose to load x tiles directly in [K, M] layout --
    this removes all 2048 PE transposes + PSUM->SBUF copies from the hot loop
    and halves x HBM traffic
  - packed_qweight is unpacked to one nibble per byte on HOST (pure bit
    marshaling): the device dequant is then 3 wide DVE ops per tile instead
    of 4 narrow ones, with no strided half-interleaving
  - on device: dequantize W shard, fuse lora via W_eff = W + B@A (bf16 PE
    matmul, staged into SBUF via scalar-engine copies), single big GEMM
    x @ W_eff.T in bf16, add bias
  - host gathers the 8 [8192, 512] outputs -> [4, 2048, 4096]
"""
import os
import sys

for _p in ("/opt/trn_rl_repo", "/root/.axon_site/_ro/trn_rl_repo"):
    if os.path.isdir(_p) and _p not in sys.path:
        sys.path.insert(0, _p)
        break

import numpy as np
import ml_dtypes

import concourse.bass as bass
import concourse.bacc as bacc
import concourse.tile as tile
import concourse.mybir as mybir

dt = mybir.dt

# problem constants (hardcoded per spec)
B_, S_, IN_F, OUT_F, RANK = 4, 2048, 4096, 4096, 16
N_CORES = 8
M = B_ * S_                    # 8192 tokens
N = OUT_F // N_CORES           # 512 out_features per core
BLOCK = 64                     # quant block size (along in_features)
OT = N // 128                  # 4 o-tiles per core
MT = M // 128                  # 64 m-tiles
KC = IN_F // 128               # 32 k-chunks


def build_program(affine: bool, c1: float, delta: float, lut_vals):
    """Build the single-core Bass program (SPMD: same program on all 8 cores)."""
    nc = bacc.Bacc("TRN2", target_bir_lowering=False, debug=False,
                   num_devices=N_CORES)

    x = nc.dram_tensor("x", [M, IN_F], dt.bfloat16, kind="ExternalInput")
    pk = nc.dram_tensor("pk", [N, IN_F], dt.uint8, kind="ExternalInput")
    wmax = nc.dram_tensor("wmax", [N, BLOCK], dt.float32, kind="ExternalInput")
    lora_a = nc.dram_tensor("lora_a", [RANK, IN_F], dt.bfloat16, kind="ExternalInput")
    lora_bt = nc.dram_tensor("lora_bt", [RANK, N], dt.bfloat16, kind="ExternalInput")
    bias = nc.dram_tensor("bias", [N], dt.float32, kind="ExternalInput")
    ident = nc.dram_tensor("ident", [128, 128], dt.bfloat16, kind="ExternalInput")
    out = nc.dram_tensor("out", [M, N], dt.float32, kind="ExternalOutput")

    with tile.TileContext(nc) as tc:
        with (
            tc.tile_pool(name="const", bufs=1) as constp,
            tc.tile_pool(name="wprep", bufs=2) as wprep,
            tc.tile_pool(name="wsb", bufs=4) as wsbp,
            tc.tile_pool(name="wt", bufs=1) as wtp,
            tc.tile_pool(name="xt", bufs=3) as xtp,
            tc.tile_pool(name="osb", bufs=3) as op_,
            tc.tile_pool(name="ps_ba", bufs=2, space="PSUM") as ps_ba,
            tc.tile_pool(name="ps_tx", bufs=2, space="PSUM") as ps_tx,
            tc.tile_pool(name="ps_out", bufs=3, space="PSUM") as ps_out,
        ):
            # ---- constants / W inputs on the sync (SP) HWDGE queue.
            # Queue is FIFO: these loads run first, then the 16 xbar
            # transpose windows of x stream behind them.
            id_sb = constp.tile([128, 128], dt.bfloat16, name="id_sb")
            nc.sync.dma_start(out=id_sb[:], in_=ident[:, :])
            a_sb = constp.tile([RANK, IN_F], dt.bfloat16, name="a_sb")
            nc.sync.dma_start(out=a_sb[:], in_=lora_a[:, :])
            bt_sb = constp.tile([RANK, N], dt.bfloat16, name="bt_sb")
            nc.sync.dma_start(out=bt_sb[:], in_=lora_bt[:, :])
            bias_sb = constp.tile([128, N], dt.float32, name="bias_sb")
            bsrc = bass.AP(bias[:].tensor, 0, [[0, 128], [1, N]])
            nc.gpsimd.dma_start(out=bias_sb[:], in_=bsrc)

            nibs, s1s = [], []
            for t in range(OT):
                nib = constp.tile([128, IN_F], dt.uint8, name=f"nib{t}")
                nc.sync.dma_start(out=nib[:], in_=pk[t * 128:(t + 1) * 128, :])
                nibs.append(nib)
                s1 = constp.tile([128, BLOCK], dt.float32, name=f"s1_{t}")
                nc.sync.dma_start(out=s1[:], in_=wmax[t * 128:(t + 1) * 128, :])
                if affine:
                    nc.vector.tensor_scalar_mul(s1[:], s1[:], float(c1))
                s1s.append(s1)

            # ---- W-prep ----
            # wt_sb[:, c*N + t*128 + oo] = W_eff[t*128+oo, c*128+p] on partition p
            # Chunk-major over 512-wide k segments so wt chunk 0 is ready
            # ~1/8 into W-prep and the main matmul loop starts almost
            # immediately.
            wt_sb = wtp.tile([128, KC * N], dt.bfloat16, name="wt_sb")
            w_sbs = []
            for t in range(OT):
                w_sb = wsbp.tile([128, IN_F], dt.bfloat16, tag="w_sb")
                w_sbs.append(w_sb)

            SW = 512
            for k0 in range(0, IN_F, SW):
                for t in range(OT):
                    nib, s1, w_sb = nibs[t], s1s[t], w_sbs[t]
                    # lora B@A for this [128 o, 512 in] segment (bf16, K=16),
                    # staged PSUM -> SBUF bf16 by the (otherwise idle) scalar
                    # engine so the DVE add below is an all-bf16 packed op
                    pba = ps_ba.tile([128, SW], dt.float32, tag="pba")
                    nc.tensor.matmul(
                        pba[:], bt_sb[:, t * 128:(t + 1) * 128],
                        a_sb[:, k0:k0 + SW],
                        start=True, stop=True,
                    )
                    # stage B@A straight into the W_eff segment; the dequant
                    # result is added in place below
                    ba_seg = w_sb[:, k0:k0 + SW]
                    nc.scalar.activation(
                        ba_seg, pba[:],
                        mybir.ActivationFunctionType.Copy, bias=0.0)

                    ns = nib[:, k0:k0 + SW]
                    tl = wprep.tile([128, SW], dt.bfloat16, tag="deq_t")
                    s_ap0 = s1[:]
                    s_b = bass.AP(
                        s_ap0.tensor, s_ap0.offset + k0 // BLOCK,
                        [list(s_ap0.ap[0]), [1, SW // BLOCK], [0, BLOCK]],
                    )
                    if affine:
                        # fused single DVE pass: (nibble + delta) * scale
                        nc.vector.scalar_tensor_tensor(
                            tl[:], ns, float(delta), s_b,
                            mybir.AluOpType.add, mybir.AluOpType.mult)
                    else:
                        # general 16-entry codebook fallback:
                        # idx -> sum_k lut[k] * (idx == k)
                        nc.vector.memset(tl[:], 0.0)
                        for k in range(16):
                            msk = wprep.tile([128, SW], dt.float32, tag="deq_msk")
                            nc.vector.tensor_scalar(
                                msk[:], ns, float(k), None,
                                mybir.AluOpType.is_equal,
                            )
                            nc.vector.tensor_scalar_mul(
                                msk[:], msk[:], float(lut_vals[k]))
                            nc.vector.tensor_tensor(
                                tl[:], tl[:], msk[:], mybir.AluOpType.add)
                        # multiply by per-block scale (broadcast 64/block)
                        nc.vector.tensor_tensor(
                            tl[:], tl[:], s_b, mybir.AluOpType.mult)
                    # add dequant onto the staged B@A in place (all-bf16 packed)
                    nc.vector.tensor_tensor(
                        ba_seg, tl[:], ba_seg, mybir.AluOpType.add)

                # transpose W_eff for the 4 k-chunks this segment completed
                for c in range(k0 // 128, (k0 + SW) // 128):
                    ptr = ps_tx.tile([128, N], dt.bfloat16, tag="ptx")
                    for t in range(OT):
                        nc.tensor.transpose(
                            ptr[:, t * 128:(t + 1) * 128],
                            w_sbs[t][:, c * 128:(c + 1) * 128],
                            id_sb[:],
                        )
                    nc.scalar.activation(
                        wt_sb[:, c * N:(c + 1) * N], ptr[:],
                        mybir.ActivationFunctionType.Copy, bias=0.0)

            # ---- main loop: xbar-transpose-load x, matmul, bias, store ----
            # 4 m-tiles per xbar instruction: the sync queue spends ~5us of
            # fixed overhead (sems + dispatch) per DMA_TRANSPOSE on top of
            # ~14ns/tile processing, so per-m-tile transposes can only supply
            # one tile per ~10us and starve the PE (8us/m-tile). Keep the
            # windows uniform: mixed-size xt tiles break the pool ring and
            # cost ~100us.
            windows = [4] * 16
            assert sum(windows) == MT
            ms0 = 0
            for MW in windows:
                xt = xtp.tile([128, KC, MW * 128], dt.bfloat16, tag="xt")
                nc.sync.dma_start_transpose(
                    out=xt[:], in_=x[ms0 * 128:(ms0 + MW) * 128, :])

                for i in range(MW):
                    ms = ms0 + i
                    po = ps_out.tile([128, N], dt.float32, tag="po")
                    for c in range(KC):
                        nc.tensor.matmul(
                            po[:],
                            xt[:, c, i * 128:(i + 1) * 128],
                            wt_sb[:, c * N:(c + 1) * N],
                            start=(c == 0), stop=(c == KC - 1),
                        )
                    o_sb = op_.tile([128, N], dt.float32, tag="o_sb")
                    nc.vector.tensor_tensor(
                        o_sb[:], po[:], bias_sb[:], mybir.AluOpType.add)
                    nc.scalar.dma_start(
                        out=out[ms * 128:(ms + 1) * 128, :], in_=o_sb[:])
                ms0 += MW

    nc.compile()
    return nc


_cache = {}


def _get_program(lut: np.ndarray):
    lut = np.asarray(lut, dtype=np.float32)
    c1 = float(lut[15] - lut[0]) / 15.0
    idx = np.arange(16, dtype=np.float32)
    affine = bool(
        np.max(np.abs(lut - (lut[0] + c1 * idx))) <= 1e-6 * max(1e-30, np.max(np.abs(lut)))
        and abs(c1) > 1e-20
    )
    delta = float(lut[0]) / c1 if affine else 0.0
    key = (affine, round(c1, 12), round(delta, 12), tuple(np.round(lut, 10).tolist()))
    if key not in _cache:
        _cache[key] = build_program(affine, c1, delta, lut.tolist())
    return _cache[key]


def make_in_maps(inputs: dict):
    x = np.ascontiguousarray(
        np.asarray(inputs["x"], dtype=np.float32).reshape(M, IN_F)
    ).astype(ml_dtypes.bfloat16)
    pk_full = np.asarray(inputs["packed_qweight"]).astype(np.uint8).reshape(-1)
    # unpack to one nibble per byte (LSB-first order, matching the reference)
    nib_full = np.empty(pk_full.size * 2, dtype=np.uint8)
    nib_full[0::2] = pk_full & 15
    nib_full[1::2] = pk_full >> 4
    nib_full = nib_full.reshape(OUT_F, IN_F)
    wmax_full = np.asarray(inputs["weight_max"], dtype=np.float32).reshape(-1)
    lora_a = np.ascontiguousarray(
        np.asarray(inputs["lora_A"], dtype=np.float32)).astype(ml_dtypes.bfloat16)
    lora_b = np.asarray(inputs["lora_B"], dtype=np.float32)
    bias_full = np.asarray(inputs["bias"], dtype=np.float32).reshape(-1)
    ident = np.eye(128, dtype=ml_dtypes.bfloat16)

    in_maps = []
    for i in range(N_CORES):
        o0, o1 = i * N, (i + 1) * N
        in_maps.append({
            "x": x,
            "pk": nib_full[o0:o1],
            "wmax": wmax_full[o0 * BLOCK: o1 * BLOCK].reshape(N, BLOCK),
            "lora_a": lora_a,
            "lora_bt": np.ascontiguousarray(lora_b[o0:o1].T).astype(ml_dtypes.bfloat16),
            "bias": bias_full[o0:o1],
            "ident": ident,
        })
    return in_maps


def kernel(**inputs) -> np.ndarray:
    from concourse.bass_utils import run_bass_kernel_spmd

    nc = _get_program(inputs["lookup_table"])
    in_maps = make_in_maps(inputs)
    res = run_bass_kernel_spmd(nc, in_maps, core_ids=list(range(N_CORES)))
    outs = [np.asarray(r["out"], dtype=np.float32) for r in res.results]
    full = np.concatenate(outs, axis=1)  # [M, OUT_F]
    return full.reshape(B_, S_, OUT_F)
